# revision 1
# baseline (speedup 1.0000x reference)
"""DigitCaps dynamic-routing kernel for 8x TRN2 NeuronCores (Bass/Tile).

Reference math (per sample b, capsule j):
    u_hat[b,j,r,o] = sum_i W[j,r,o,i] * u[b,r,i]
    b_ij = 0
    3 iterations:
        c = softmax_r(b_ij); s = sum_r c*u_hat; v = squash(s)
        b_ij += sum_o u_hat*v  (first 2 iters)
    return v  [B, J, O]

Sharding: J (166 -> 168 padded) split across 8 cores, 21 capsules each.
Every core sees the full batch B=128 (partition dim for batch-parallel ops).

Per-core algorithm (u_hat never materialized; all heavy contractions on PE):
    s0    = sum_{r,i} (W/R)[j,r,o,i] u[b,r,i]            (PE, k=(r,i) chunks)
    v0    = squash(s0)                                    (tiny DVE/ACT)
    iter t=1,2:
      T1[b,(r,i)] = sum_o v[b,j,o] W[j,r,o,i]            (PE, k=o, per j)
      a[b,j,r]    = sum_i T1*u                            (DVE mul+reduce)
      b_ij        = a1 (+ a2 on t=2, via DRAM scratch)
      e = exp(b_ij) in r-on-partition layout              (DMA-transpose + ACT)
      Z = sum_r e                                         (PE ones-matmul)
      cur[(r),(i,b)] = e * u                              (DVE)
      sT[o,(j,b)] = sum_{r,i} W*cur                       (PE, k=r chunks)
      v = squash(sT / Z)                                  (o-partition layout)
"""

import sys

sys.path.insert(0, "/opt/trn_rl_repo")

import numpy as np
import ml_dtypes
from contextlib import ExitStack

import concourse.bacc as bacc
import concourse.bass as bass
import concourse.tile as tile
from concourse import mybir
from concourse.masks import make_identity
from concourse.bass_utils import run_bass_kernel_spmd

F32 = mybir.dt.float32
BF16 = mybir.dt.bfloat16
AX = mybir.AxisListType
ACT_F = mybir.ActivationFunctionType

B, J, R, O, I = 128, 166, 864, 8, 4
NCORES = 8
JL = 21          # capsules per core (166 padded to 168)
RP = 896         # R padded to 7*128
C32 = 27         # (r32, i4)=128 contraction chunks over real R=864
C128 = 7         # r chunks of 128 over RP
JO = JL * O      # 168
JB = JL * B      # 2688
NZB = 6          # Z col-blocks of 448 (6*448 = 2688)
ZW = 448


def build_nc(repeat=1):
    nc = bacc.Bacc(
        "TRN2", target_bir_lowering=False, debug=False, enable_asserts=False
    )

    d_uT_ri = nc.dram_tensor("uT_ri", [C32, 128, 128], BF16, kind="ExternalInput")
    d_Wm_ri = nc.dram_tensor("Wm_ri", [C32, 128, JO], BF16, kind="ExternalInput")
    d_Wr = nc.dram_tensor("Wr", [C128, 128, I * JL * O], BF16, kind="ExternalInput")
    d_W2 = nc.dram_tensor("W2", [JL, 8, I * RP], BF16, kind="ExternalInput")
    d_u_b = nc.dram_tensor("u_b", [128, RP * I], BF16, kind="ExternalInput")
    d_uTr = nc.dram_tensor("uTr", [C128, 128, I * B], BF16, kind="ExternalInput")
    d_out = nc.dram_tensor("v_out", [8, JL, B], F32, kind="ExternalOutput")

    with tile.TileContext(nc) as tc:
        for _ in range(repeat):
            _body(tc, d_uT_ri, d_Wm_ri, d_Wr, d_W2, d_u_b, d_uTr, d_out)
    nc.compile()
    return nc


def _body(tc, d_uT_ri, d_Wm_ri, d_Wr, d_W2, d_u_b, d_uTr, d_out):
    nc = tc.nc
    es = ExitStack()
    const = es.enter_context(tc.tile_pool(name="const", bufs=1))
    misc = es.enter_context(tc.tile_pool(name="misc", bufs=1))
    vpool = es.enter_context(tc.tile_pool(name="vpool", bufs=2))
    stream = es.enter_context(tc.tile_pool(name="stream", bufs=2))
    prodp = es.enter_context(tc.tile_pool(name="prodp", bufs=2))
    curp = es.enter_context(tc.tile_pool(name="curp", bufs=8))
    ep = es.enter_context(tc.tile_pool(name="ep", bufs=1))
    dramp = es.enter_context(tc.tile_pool(name="dramp", bufs=1, space="DRAM"))
    psum_big = es.enter_context(tc.tile_pool(name="psum_big", bufs=1, space="PSUM"))
    psum_sm = es.enter_context(tc.tile_pool(name="psum_sm", bufs=1, space="PSUM"))

    with es:
        # ---------------- constants / persistent loads ----------------
        identity = const.tile([128, 128], F32)
        make_identity(nc, identity[:])
        ones_r = const.tile([128, 1], BF16)
        nc.vector.memset(ones_r[:], 1.0)
        ones_r96 = const.tile([128, 1], BF16)
        nc.vector.memset(ones_r96[:], 0.0)
        nc.vector.memset(ones_r96[:96, :], 1.0)
        ones_o = const.tile([8, 1], BF16)
        nc.vector.memset(ones_o[:], 1.0)

        u_b = const.tile([128, RP * I], BF16)
        nc.sync.dma_start(u_b[:], d_u_b[:])
        uTr_all = const.tile([128, C128 * I * B], BF16)
        nc.sync.dma_start(
            uTr_all[:].rearrange("p (c n) -> p c n", c=C128),
            d_uTr[:].transpose([1, 0, 2]),
        )
        uTr = [uTr_all[:, c * I * B : (c + 1) * I * B] for c in range(C128)]
        wr_all = const.tile([128, C128 * I * JL * O], BF16)
        nc.sync.dma_start(
            wr_all[:].rearrange("p (c n) -> p c n", c=C128),
            d_Wr[:].transpose([1, 0, 2]),
        )
        wr = [wr_all[:, c * I * JL * O : (c + 1) * I * JL * O] for c in range(C128)]

        a1_dram = dramp.tile([JL, 128, RP], BF16)
        g_dram = dramp.tile([1, JB], F32)

        # ---------------- s0: full (r,i) contraction, b-partition out -------
        with tc.tile_pool(name="s0p", bufs=2) as s0p:
            s0ps = psum_sm.tile([128, 512], F32, tag="small", name="s0ps")
            GB = 9
            for g in range(C32 // GB):
                a = s0p.tile([128, GB * 128], BF16, tag="ut", name=f"ut{g}")
                nc.sync.dma_start(
                    a[:].rearrange("p (c n) -> p c n", c=GB),
                    d_uT_ri[g * GB : (g + 1) * GB].transpose([1, 0, 2]),
                )
                b = s0p.tile([128, GB * JO], BF16, tag="wm", name=f"wm{g}")
                nc.sync.dma_start(
                    b[:].rearrange("p (c n) -> p c n", c=GB),
                    d_Wm_ri[g * GB : (g + 1) * GB].transpose([1, 0, 2]),
                )
                for cc in range(GB):
                    c = g * GB + cc
                    nc.tensor.matmul(
                        s0ps[:, :JO],
                        lhsT=a[:, cc * 128 : (cc + 1) * 128],
                        rhs=b[:, cc * JO : (cc + 1) * JO],
                        start=(c == 0),
                        stop=(c == C32 - 1),
                    )
            s0_sb = s0p.tile([128, JO], F32)
            nc.scalar.copy(s0_sb[:], s0ps[:, :JO])

            # squash in b-layout (tiny)
            sqb = s0p.tile([128, JO], F32)
            nc.vector.tensor_mul(sqb[:], s0_sb[:], s0_sb[:])
            n2b = s0p.tile([128, JL], F32)
            nc.vector.reduce_sum(
                n2b[:], sqb[:].rearrange("p (j o) -> p j o", j=JL), axis=AX.X
            )
            rtb = s0p.tile([128, JL], F32)
            nc.scalar.sqrt(rtb[:], n2b[:])
            nc.vector.tensor_scalar_add(n2b[:], n2b[:], 1.0)
            nc.vector.reciprocal(n2b[:], n2b[:])
            nc.vector.tensor_mul(rtb[:], rtb[:], n2b[:])  # f = sqrt(n2)/(1+n2)
            v0b = s0p.tile([128, JO], F32)
            nc.vector.tensor_mul(
                v0b[:].rearrange("p (j o) -> p j o", j=JL),
                s0_sb[:].rearrange("p (j o) -> p j o", j=JL),
                rtb[:].unsqueeze(2).broadcast_to([128, JL, O]),
            )
            # transpose v0 to o-partition layout -> v_prev (bf16)
            v_prev = vpool.tile([8, JB], BF16, tag="vbf")
            jj = 0
            while jj < JL:
                take = min(4, JL - jj)
                tps = psum_sm.tile([128, 512], F32, tag="small", name="tps")
                for q in range(take):
                    nc.tensor.transpose(
                        tps[0:8, q * 128 : (q + 1) * 128],
                        v0b[:].rearrange("p (j o) -> p j o", j=JL)[:, jj + q, :],
                        identity[:],
                    )
                nc.scalar.copy(
                    v_prev[:, jj * 128 : (jj + take) * 128], tps[0:8, : take * 128]
                )
                jj += take

        # ---------------- routing iterations ----------------
        for t in (1, 2):
            e_all = ep.tile([128, C128 * JB], BF16, tag="e_all", name=f"e_all_{t}")
            e_t = [e_all[:, c * JB : (c + 1) * JB] for c in range(C128)]

            # ---- a-phase: per-j T1 matmul + mul/reduce + transpose out ----
            WB = 3  # W2 j-batch
            w2_tiles = {}
            for j in range(JL):
                if j % WB == 0:
                    w2 = stream.tile([8, WB * I * RP], BF16, tag="w2", name="w2", bufs=1)
                    nc.sync.dma_start(
                        w2[:].rearrange("p (j n) -> p j n", j=WB),
                        d_W2[j : j + WB].transpose([1, 0, 2]),
                    )
                    w2_tiles[j // WB] = w2
                w2 = w2_tiles[j // WB]
                w2v = w2[:].rearrange("p (j i r) -> p j i r", j=WB, i=I)[:, j % WB]
                t1 = psum_big.tile([128, 512 * C128], F32, tag="T1", name="t1")
                lhs = v_prev[:, j * 128 : (j + 1) * 128]
                for c in range(C128):
                    mv = w2v[:, :, c * 128 : (c + 1) * 128].transpose([0, 2, 1])
                    nc.tensor.matmul(
                        t1[:, c * 512 : (c + 1) * 512],
                        lhsT=lhs,
                        rhs=mv,
                        start=True,
                        stop=True,
                    )
                a_j = stream.tile([128, RP], F32, tag="aj", name="a_j")
                t1bf = prodp.tile([128, 512 * C128], BF16, tag="t1bf", name="t1bf")
                if j % 3 == 0:
                    # direct path: DVE drains PSUM via the fused mul
                    nc.vector.tensor_mul(t1bf[:], t1[:], u_b[:])
                else:
                    # ACT drains PSUM; mul in bf16 on DVE/GPSIMD
                    eng = nc.vector if j % 3 == 1 else nc.gpsimd
                    nc.scalar.copy(t1bf[:], t1[:])
                    eng.tensor_mul(t1bf[:], t1bf[:], u_b[:])
                nc.vector.reduce_sum(
                    a_j[:],
                    t1bf[:].rearrange("p (r i) -> p r i", i=I),
                    axis=AX.X,
                )
                a_bf = stream.tile([128, RP], BF16, tag="abf", name="a_bf")
                if t == 1:
                    nc.scalar.copy(a_bf[:], a_j[:])
                    nc.gpsimd.dma_start(a1_dram[j], a_bf[:])
                else:
                    a1_f = stream.tile([128, RP], BF16, tag="a1f", name="a1_f")
                    nc.gpsimd.dma_start(a1_f[:], a1_dram[j])
                    nc.vector.tensor_add(a_bf[:], a_j[:], a1_f[:])
                nc.scalar.dma_start_transpose(
                    e_all[:]
                    .rearrange("p (c j b) -> p c j b", c=C128, j=JL)[
                        :, :, j, :
                    ],
                    a_bf[:],
                )

            # ---- e-phase: exp in place; Z = sum_r e; rZ = 1/Z ----
            for c in range(C128):
                nc.scalar.activation(e_t[c], e_t[c], ACT_F.Exp)
            rZ = misc.tile([1, JB], F32, tag="rZ", name=f"rZ_{t}")
            for n in range(NZB):
                zs = psum_sm.tile([128, 512], F32, tag="small", name="zs")
                for c in range(C128):
                    ones = ones_r96 if c == C128 - 1 else ones_r
                    nc.tensor.matmul(
                        zs[0:1, :ZW],
                        lhsT=ones[:],
                        rhs=e_t[c][:, n * ZW : (n + 1) * ZW],
                        start=(c == 0),
                        stop=(c == C128 - 1),
                    )
                nc.vector.reciprocal(zs[0:1, :ZW], zs[0:1, :ZW])
                nc.scalar.copy(rZ[:, n * ZW : (n + 1) * ZW], zs[0:1, :ZW])

            # ---- s-phase: cur = e*u per (c,j); sT = sum_{r,i} W*cur ----
            sT = misc.tile([8, JB], F32, tag="sT", name=f"sT_{t}")
            jj = 0
            while jj < JL:
                take = min(4, JL - jj)
                sps = psum_sm.tile([128, 512], F32, tag="small", name="sps")
                for q in range(take):
                    jx = jj + q
                    curs = []
                    for c in range(C128):
                        cur = curp.tile([128, I * B], BF16, tag="cur", name="cur")
                        ceng = nc.gpsimd if c >= 5 else nc.vector
                        ceng.tensor_mul(
                            cur[:].rearrange("p (i b) -> p i b", i=I),
                            e_t[c][:, jx * 128 : (jx + 1) * 128]
                            .unsqueeze(1)
                            .broadcast_to([128, I, B]),
                            uTr[c][:].rearrange("p (i b) -> p i b", i=I),
                        )
                        curs.append(cur)
                    for c in range(C128):
                        for i in range(I):
                            nc.tensor.matmul(
                                sps[0:8, q * 128 : (q + 1) * 128],
                                lhsT=wr[c][:, (i * JL + jx) * 8 : (i * JL + jx) * 8 + 8],
                                rhs=curs[c][:].rearrange("p (i b) -> p i b", i=I)[
                                    :, i, :
                                ],
                                start=(c == 0 and i == 0),
                                stop=(c == C128 - 1 and i == I - 1),
                            )
                nc.scalar.copy(sT[:, jj * 128 : (jj + take) * 128], sps[0:8, : take * 128])
                jj += take

            # ---- squash phase in o-partition layout ----
            # v = sT * g,  g = rZ*sqrt(q)/(1+q),  q = rZ^2 * m2,  m2 = sum_o sT^2
            sq = misc.tile([8, JB], BF16, tag="sq", name=f"sq_{t}")
            nc.scalar.square(sq[:], sT[:])
            m2 = misc.tile([1, JB], F32, tag="m2", name=f"m2_{t}")
            for n in range(NZB):
                ns = psum_sm.tile([128, 512], F32, tag="small", name="ns")
                nc.tensor.matmul(
                    ns[0:1, :ZW],
                    lhsT=ones_o[:],
                    rhs=sq[:, n * ZW : (n + 1) * ZW],
                    start=True,
                    stop=True,
                )
                nc.scalar.copy(m2[:, n * ZW : (n + 1) * ZW], ns[0:1, :ZW])
            tmp = misc.tile([1, JB], F32, tag="tmp", name=f"tmp_{t}")
            nc.vector.tensor_mul(tmp[:], rZ[:], rZ[:])
            nc.vector.tensor_mul(m2[:], m2[:], tmp[:])  # q
            nc.scalar.sqrt(tmp[:], m2[:])               # sqrt(q)
            nc.vector.tensor_scalar_add(m2[:], m2[:], 1.0)
            nc.vector.reciprocal(m2[:], m2[:])          # 1/(1+q)
            nc.vector.tensor_mul(tmp[:], tmp[:], m2[:])
            nc.vector.tensor_mul(tmp[:], tmp[:], rZ[:])  # g
            g8 = misc.tile([8, JB], BF16, tag="g8", name=f"g8_{t}")
            nc.gpsimd.dma_start(g_dram[:], tmp[:])
            nc.gpsimd.dma_start(
                g8[:], g_dram[:].squeeze(0).unsqueeze(0).broadcast_to([8, JB])
            )

            if t == 1:
                v_prev = vpool.tile([8, JB], BF16, tag="vbf", name="v1")
                nc.vector.tensor_mul(v_prev[:], sT[:], g8[:])
            else:
                v_fin = misc.tile([8, JB], F32, tag="tmp", name="v_fin")
                nc.vector.tensor_mul(v_fin[:], sT[:], g8[:])
                nc.gpsimd.dma_start(d_out[:].rearrange("o j b -> o (j b)"), v_fin[:])


# ---------------------------------------------------------------------------
# Host side
# ---------------------------------------------------------------------------

_NC_CACHE = None


def _get_nc():
    global _NC_CACHE
    if _NC_CACHE is None:
        _NC_CACHE = build_nc()
    return _NC_CACHE


def _host_prep(u, W):
    """Build per-core input maps. u: [B,R,I] f32; W: [1,J,R,O,I] f32."""
    bf = ml_dtypes.bfloat16
    u = np.ascontiguousarray(u, dtype=np.float32)
    Wq = np.ascontiguousarray(W.reshape(J, R, O, I), dtype=np.float32)

    up = np.zeros((B, RP, I), np.float32)
    up[:, :R] = u
    u_b = np.ascontiguousarray(up.reshape(128, RP * I).astype(bf))
    uT = np.ascontiguousarray(u.transpose(1, 2, 0))        # [864, 4, 128]
    uT_ri = np.ascontiguousarray(uT.reshape(C32, 128, 128).astype(bf))
    uTp = np.ascontiguousarray(up.transpose(1, 2, 0))      # [896, 4, 128]
    uTr = np.ascontiguousarray(uTp.reshape(C128, 128, I * B).astype(bf))

    in_maps = []
    for k in range(NCORES):
        j0 = k * JL
        Wk = np.zeros((JL, R, O, I), np.float32)
        real = min(JL, max(0, J - j0))
        if real > 0:
            Wk[:real] = Wq[j0 : j0 + real]
        Wkp = np.zeros((JL, RP, O, I), np.float32)
        Wkp[:, :R] = Wk

        wm_ri = np.ascontiguousarray(
            (Wk / float(R)).transpose(1, 3, 0, 2).reshape(C32, 128, JO).astype(bf)
        )
        wr = np.ascontiguousarray(
            Wkp.transpose(1, 3, 0, 2).reshape(C128, 128, I * JL * O).astype(bf)
        )
        w2 = np.ascontiguousarray(
            Wkp.transpose(0, 2, 3, 1).reshape(JL, 8, I * RP).astype(bf)
        )
        in_maps.append(
            {
                "uT_ri": uT_ri,
                "Wm_ri": wm_ri,
                "Wr": wr,
                "W2": w2,
                "u_b": u_b,
                "uTr": uTr,
            }
        )
    return in_maps


def run_cores(u, W, trace=False):
    nc = _get_nc()
    in_maps = _host_prep(u, W)
    res = run_bass_kernel_spmd(
        nc, in_maps, core_ids=list(range(NCORES)), trace=trace
    )
    return res


def kernel(u, W):
    res = run_cores(u, W, trace=False)
    parts = []
    for k in range(NCORES):
        vk = res.results[k]["v_out"]          # [8, JL, 128] (o, j, b)
        parts.append(np.ascontiguousarray(vk.transpose(2, 1, 0)))  # [B, JL, O]
    full = np.concatenate(parts, axis=1)[:, :J, :]
    return np.ascontiguousarray(full.astype(np.float32))



# revision 3
# speedup vs baseline: 2.3393x; 2.3393x over previous
"""DigitCaps dynamic-routing kernel for 8x TRN2 NeuronCores (Bass/Tile).

Same math as v1, restructured for pipeline overlap:
  - a-phase T1 PSUM drained in 1024-col chunks, rotated across ACT/DVE/Pool
  - i-major T1 layout; i-sum (and +a1 on t=2) via gpsimd accumulate-DMAs
  - exp per-j in b-partition layout on ACT
  - e transposed to r-layout via SP-dispatched DMA transpose
  - cur muls on DVE only; s-phase matmuls in baseline [8,128] form
  - Z via PE ones-matmul on e_all; squash as v1

Sharding: J (166 -> 168 padded) split across 8 cores, 21 capsules each.
"""

import sys

sys.path.insert(0, "/opt/trn_rl_repo")

import numpy as np
import ml_dtypes
from contextlib import ExitStack

import concourse.bacc as bacc
import concourse.bass as bass
import concourse.tile as tile
from concourse import mybir
from concourse.masks import make_identity
from concourse.bass_utils import run_bass_kernel_spmd

F32 = mybir.dt.float32
BF16 = mybir.dt.bfloat16
AX = mybir.AxisListType
ACT_F = mybir.ActivationFunctionType
ALU = mybir.AluOpType

B, J, R, O, I = 128, 166, 864, 8, 4
NCORES = 8
JL = 21          # capsules per core (166 padded to 168)
RP = 896         # R padded to 7*128
C32 = 27         # (r32, i4)=128 contraction chunks over real R=864
C128 = 7         # r chunks of 128 over RP
JO = JL * O      # 168
JB = JL * B      # 2688
NZB = 6          # Z col-blocks of 448 (6*448 = 2688)
ZW = 448
IR = I * RP      # 3584, i-major (i, r) free layout

# engine-assignment knobs (tuned against TimelineSim)
DRAIN_DVE = 0       # j % DRAIN_DVE == DRAIN_DVE-1 -> DVE-fused drain, else ACT
ISUM_POOL = 0       # j % ISUM_POOL == 0 -> Pool accum-DMA i-sum, else DVE adds
CUR_POOL = 0        # j % CUR_POOL == CUR_POOL-1 -> Pool cur muls (0 = never)
YMUL_POOL = 5       # j % YMUL_POOL == YMUL_POOL-1 -> Pool y-mul (0 = never)


def build_nc(repeat=1):
    nc = bacc.Bacc(
        "TRN2", target_bir_lowering=False, debug=False, enable_asserts=False
    )

    d_uT_ri = nc.dram_tensor("uT_ri", [C32, 128, 128], BF16, kind="ExternalInput")
    d_Wm_ri = nc.dram_tensor("Wm_ri", [C32, 128, JO], BF16, kind="ExternalInput")
    d_Wr = nc.dram_tensor("Wr", [C128, 128, I * JL * O], BF16, kind="ExternalInput")
    # W2 in (o, i, r) per j: [JL, 8, IR] with (i major, r minor)
    d_W2 = nc.dram_tensor("W2", [JL, 8, IR], BF16, kind="ExternalInput")
    # u in i-major b-layout: [b, (i, r)]
    d_u_ib = nc.dram_tensor("u_ib", [128, IR], BF16, kind="ExternalInput")
    d_uTr = nc.dram_tensor("uTr", [C128, 128, I * B], BF16, kind="ExternalInput")
    d_out = nc.dram_tensor("v_out", [128, JL * O], F32, kind="ExternalOutput")

    with tile.TileContext(nc) as tc:
        for _ in range(repeat):
            _body(tc, d_uT_ri, d_Wm_ri, d_Wr, d_W2, d_u_ib, d_uTr, d_out)
    nc.compile()
    return nc


def _body(tc, d_uT_ri, d_Wm_ri, d_Wr, d_W2, d_u_ib, d_uTr, d_out):
    nc = tc.nc
    es = ExitStack()
    const = es.enter_context(tc.tile_pool(name="const", bufs=1))
    misc = es.enter_context(tc.tile_pool(name="misc", bufs=1))
    vpool = es.enter_context(tc.tile_pool(name="vpool", bufs=2))
    w2s = es.enter_context(tc.tile_pool(name="w2s", bufs=2))
    ypool = es.enter_context(tc.tile_pool(name="ypool", bufs=3))
    ycpool = es.enter_context(tc.tile_pool(name="ycpool", bufs=5))
    apool = es.enter_context(tc.tile_pool(name="apool", bufs=6))
    epool = es.enter_context(tc.tile_pool(name="epool", bufs=6))
    curp = es.enter_context(tc.tile_pool(name="curp", bufs=16))
    dramp = es.enter_context(tc.tile_pool(name="dramp", bufs=1, space="DRAM"))
    psA = es.enter_context(tc.tile_pool(name="psA", bufs=3, space="PSUM"))
    psB = es.enter_context(tc.tile_pool(name="psB", bufs=2, space="PSUM"))

    with es:
        # ---------------- constants / persistent loads ----------------
        identity = const.tile([128, 128], F32)
        make_identity(nc, identity[:])
        ones_r = const.tile([128, 1], BF16)
        nc.vector.memset(ones_r[:], 1.0)
        ones_r96 = const.tile([128, 1], BF16)
        nc.vector.memset(ones_r96[:], 0.0)
        nc.vector.memset(ones_r96[:96, :], 1.0)
        ones_o = const.tile([8, 1], BF16)
        nc.vector.memset(ones_o[:], 1.0)
        ones18 = const.tile([1, 8], BF16)
        nc.vector.memset(ones18[:], 1.0)

        u_ib = const.tile([128, IR], BF16)
        nc.gpsimd.dma_start(u_ib[:], d_u_ib[:])
        uTr_all = const.tile([128, C128 * I * B], BF16)
        nc.gpsimd.dma_start(
            uTr_all[:].rearrange("p (c n) -> p c n", c=C128),
            d_uTr[:].transpose([1, 0, 2]),
        )
        uTr = [uTr_all[:, c * I * B : (c + 1) * I * B] for c in range(C128)]
        wr_all = const.tile([128, C128 * I * JL * O], BF16)
        nc.gpsimd.dma_start(
            wr_all[:].rearrange("p (c n) -> p c n", c=C128),
            d_Wr[:].transpose([1, 0, 2]),
        )
        wr = [wr_all[:, c * I * JL * O : (c + 1) * I * JL * O] for c in range(C128)]

        g_dram = dramp.tile([1, JB], F32)
        a1_dram = dramp.tile([JL, 128, RP], BF16)

        # ---------------- s0: full (r,i) contraction, b-partition out -------
        _W2_PREFETCH = True
        with tc.tile_pool(name="s0p", bufs=2) as s0p:
            s0ps = psB.tile([128, 512], F32, tag="psB", name="s0ps")
            GB = 9
            for g in range(C32 // GB):
                a = s0p.tile([128, GB * 128], BF16, tag="ut", name=f"ut{g}")
                nc.sync.dma_start(
                    a[:].rearrange("p (c n) -> p c n", c=GB),
                    d_uT_ri[g * GB : (g + 1) * GB].transpose([1, 0, 2]),
                )
                b = s0p.tile([128, GB * JO], BF16, tag="wm", name=f"wm{g}")
                nc.gpsimd.dma_start(
                    b[:].rearrange("p (c n) -> p c n", c=GB),
                    d_Wm_ri[g * GB : (g + 1) * GB].transpose([1, 0, 2]),
                )
                for cc in range(GB):
                    c = g * GB + cc
                    nc.tensor.matmul(
                        s0ps[:, :JO],
                        lhsT=a[:, cc * 128 : (cc + 1) * 128],
                        rhs=b[:, cc * JO : (cc + 1) * JO],
                        start=(c == 0),
                        stop=(c == C32 - 1),
                    )
            s0_sb = s0p.tile([128, JO], F32)
            nc.scalar.copy(s0_sb[:], s0ps[:, :JO])

            # squash in b-layout (tiny)
            sqb = s0p.tile([128, JO], F32)
            nc.vector.tensor_mul(sqb[:], s0_sb[:], s0_sb[:])
            n2b = s0p.tile([128, JL], F32)
            nc.vector.reduce_sum(
                n2b[:], sqb[:].rearrange("p (j o) -> p j o", j=JL), axis=AX.X
            )
            rtb = s0p.tile([128, JL], F32)
            nc.scalar.sqrt(rtb[:], n2b[:])
            nc.vector.tensor_scalar_add(n2b[:], n2b[:], 1.0)
            nc.vector.reciprocal(n2b[:], n2b[:])
            nc.vector.tensor_mul(rtb[:], rtb[:], n2b[:])  # f = sqrt(n2)/(1+n2)
            v0b = s0p.tile([128, JO], F32)
            nc.vector.tensor_mul(
                v0b[:].rearrange("p (j o) -> p j o", j=JL),
                s0_sb[:].rearrange("p (j o) -> p j o", j=JL),
                rtb[:].unsqueeze(2).broadcast_to([128, JL, O]),
            )
            # transpose v0 to o-partition layout -> per-group v tiles (bf16)
            v_prev = []
            jj = 0
            while jj < JL:
                take = min(4, JL - jj)
                tps = psB.tile([128, 512], F32, tag="psB", name="tps")
                for q in range(take):
                    nc.tensor.transpose(
                        tps[0:8, q * 128 : (q + 1) * 128],
                        v0b[:].rearrange("p (j o) -> p j o", j=JL)[:, jj + q, :],
                        identity[:],
                    )
                vg = vpool.tile([8, 512], BF16, tag=f"vg{jj // 4}", name="vg")
                nc.scalar.copy(vg[:, : take * 128], tps[0:8, : take * 128])
                v_prev.append(vg)
                jj += take

        # ---------------- routing iterations ----------------
        w2_tiles = {}

        def ensure_w2(j):
            b = j // 2
            if b in w2_tiles:
                return w2_tiles[b]
            nb = min(2, JL - j)
            w2 = w2s.tile([8, 2 * IR], BF16, tag="w2", name="w2")
            nc.sync.dma_start(
                w2[:].rearrange("p (j n) -> p j n", j=2)[:, :nb],
                d_W2[j : j + nb].transpose([1, 0, 2]),
            )
            w2_tiles[b] = w2
            return w2

        for t in (1, 2):
            Zt = misc.tile([128, JL], F32, tag="Zt", name=f"Zt_{t}")
            if t == 1:
                v_next = []
                vfin_t = None
            else:
                vfin_t = misc.tile([128, JO], F32, tag="vfin", name="v_fin")
            sps = None
            yc_t = {}
            acc_t = {}
            eT_t = {}
            zsg_t = {}

            def stage_A(j):
                """W2 stream, T1 matmuls, ACT chunk drains."""
                w2 = ensure_w2(j - (j % 2))
                if j % 2 == 0:
                    w2_tiles.pop((j - 2) // 2, None)
                    ensure_w2(j + 2) if j + 2 < JL else None
                w2v = w2[:].rearrange("p (j n) -> p j n", j=2)[:, j % 2]
                lhs = v_prev[j // 4][:, (j % 4) * 128 : (j % 4 + 1) * 128]
                yc = ycpool.tile([128, IR], BF16, tag="yc", name="yc")
                yc_t[j] = yc
                off = 0
                for w in (1024, 1024, 1024, 512):
                    t1 = psA.tile([128, 1024], F32, tag="t1", name="t1")
                    for h in range(0, w, 512):
                        nc.tensor.matmul(
                            t1[:, h : h + 512],
                            lhsT=lhs,
                            rhs=w2v[:, off + h : off + h + 512],
                            start=True,
                            stop=True,
                        )
                    nc.scalar.copy(yc[:, off : off + w], t1[:, :w])
                    off += w

            def stage_M(j):
                """y = T1*u, i-sum adds, a1 store/load."""
                yc = yc_t[j]
                ym_eng = (
                    nc.gpsimd
                    if (YMUL_POOL and j % YMUL_POOL == YMUL_POOL - 1)
                    else nc.vector
                )
                ym_eng.tensor_mul(yc[:], yc[:], u_ib[:])
                a_acc = apool.tile([128, RP], BF16, tag="acc", name="acc")
                acc_t[j] = a_acc
                nc.vector.tensor_add(
                    yc[:, : 2 * RP], yc[:, : 2 * RP], yc[:, 2 * RP :]
                )
                if t == 1:
                    nc.vector.tensor_add(a_acc[:], yc[:, :RP], yc[:, RP : 2 * RP])
                    nc.gpsimd.dma_start(a1_dram[j], a_acc[:])
                else:
                    nc.vector.tensor_add(
                        yc[:, :RP], yc[:, :RP], yc[:, RP : 2 * RP]
                    )
                    a1l = apool.tile([128, RP], BF16, tag="a1l", name="a1l")
                    nc.gpsimd.dma_start(a1l[:], a1_dram[j])
                    nc.vector.tensor_add(a_acc[:], yc[:, :RP], a1l[:])

            def stage_E(j):
                """exp, transpose to r-layout, Z matmuls, rZ."""
                a_acc = acc_t.pop(j)
                e_b = apool.tile([128, RP], BF16, tag="eb", name="e_b")
                # Z accumulates sum_r e (incl. 32 pad cols of exp(0)=1,
                # corrected at the group tail)
                nc.scalar.activation(
                    e_b[:], a_acc[:], ACT_F.Exp, accum_out=Zt[:, j : j + 1]
                )
                e_T = epool.tile([128, C128 * 128], BF16, tag="eT", name="e_T")
                eT_t[j] = e_T
                nc.sync.dma_start_transpose(
                    e_T[:].rearrange("p (c b) -> p c b", c=C128), e_b[:]
                )

            def stage_C(j):
                """cur muls, s-phase matmuls, group tail squash."""
                nonlocal sps
                e_T = eT_t.pop(j)
                if j % 4 == 0:
                    sps = psB.tile([128, 512], F32, tag="psB", name="sps")
                q = j % 4
                cur_eng = (
                    nc.gpsimd
                    if (CUR_POOL and j % CUR_POOL == CUR_POOL - 1)
                    else nc.vector
                )
                curs = []
                for c in range(C128):
                    cur = curp.tile([128, I * B], BF16, tag="cur", name="cur")
                    cur_eng.tensor_mul(
                        cur[:].rearrange("p (i b) -> p i b", i=I),
                        e_T[:, c * 128 : (c + 1) * 128]
                        .unsqueeze(1)
                        .broadcast_to([128, I, B]),
                        uTr[c][:].rearrange("p (i b) -> p i b", i=I),
                    )
                    curs.append(cur)
                for c in range(C128):
                    for i in range(I):
                        nc.tensor.matmul(
                            sps[:, q * 8 : (q + 1) * 8],
                            lhsT=curs[c][:].rearrange("p (i b) -> p i b", i=I)[:, i, :],
                            rhs=wr[c][:, (i * JL + j) * 8 : (i * JL + j) * 8 + 8],
                            start=(c == 0 and i == 0),
                            stop=(c == C128 - 1 and i == I - 1),
                        )
                if q == 3 or j == JL - 1:
                    take = q + 1
                    j0 = j - q
                    wd = take * 8
                    # s_b [128, take*8] from psum; squash in b-layout
                    sb = misc.tile([128, JO], BF16, tag="sb", name=f"sb_{t}")
                    sbs = sb[:, j0 * 8 : j0 * 8 + wd]
                    nc.scalar.copy(sbs, sps[:, :wd])
                    sqb = apool.tile([128, 32], BF16, tag="sqb", name="sqb")
                    nc.scalar.square(sqb[:, :wd], sps[:, :wd])
                    m2b = apool.tile([128, 4], F32, tag="m2b", name="m2b")
                    nc.vector.reduce_sum(
                        m2b[:, :take],
                        sqb[:, :wd].rearrange("p (j o) -> p j o", o=O),
                        axis=AX.X,
                    )
                    gb = apool.tile([128, 4], F32, tag="gb", name="gb")
                    tb = apool.tile([128, 4], F32, tag="tb", name="tb")
                    zc = apool.tile([128, 4], F32, tag="zc", name="zc")
                    # rZ = 1/(Z - 32) ; pad cols contribute exp(0)=1 each
                    nc.vector.tensor_scalar_add(
                        zc[:, :take], Zt[:, j0 : j0 + take], -32.0
                    )
                    nc.vector.reciprocal(zc[:, :take], zc[:, :take])
                    nc.vector.tensor_mul(tb[:, :take], zc[:, :take], zc[:, :take])
                    nc.vector.tensor_mul(
                        m2b[:, :take], m2b[:, :take], tb[:, :take]
                    )  # q
                    nc.scalar.sqrt(tb[:, :take], m2b[:, :take])  # sqrt(q)
                    nc.vector.tensor_scalar_add(m2b[:, :take], m2b[:, :take], 1.0)
                    nc.vector.reciprocal(m2b[:, :take], m2b[:, :take])  # 1/(1+q)
                    nc.vector.tensor_mul(tb[:, :take], tb[:, :take], m2b[:, :take])
                    nc.vector.tensor_mul(
                        gb[:, :take], tb[:, :take], zc[:, :take]
                    )  # g
                    if t == 1:
                        # v_b = s_b * g  then transpose to o-layout group tile
                        vb = apool.tile([128, 32], F32, tag="vb", name="vb")
                        nc.vector.tensor_mul(
                            vb[:, :wd].rearrange("p (j o) -> p j o", o=O),
                            sbs.rearrange("p (j o) -> p j o", o=O),
                            gb[:, :take].unsqueeze(2).broadcast_to(
                                [128, take, O]
                            ),
                        )
                        tps = psB.tile([128, 512], F32, tag="psB", name="tpv")
                        for qq in range(take):
                            nc.tensor.transpose(
                                tps[0:8, qq * 128 : (qq + 1) * 128],
                                vb[:, qq * 8 : (qq + 1) * 8],
                                identity[:],
                            )
                        vg = vpool.tile([8, 512], BF16, tag=f"vg{j0 // 4}",
                                        name="vgn")
                        nc.scalar.copy(vg[:, : take * 128], tps[0:8, : take * 128])
                        v_next.append(vg)
                    else:
                        vf = vfin_t
                        nc.vector.tensor_mul(
                            vf[:, j0 * 8 : j0 * 8 + wd].rearrange(
                                "p (j o) -> p j o", o=O
                            ),
                            sbs.rearrange("p (j o) -> p j o", o=O),
                            gb[:, :take].unsqueeze(2).broadcast_to(
                                [128, take, O]
                            ),
                        )
                        nc.gpsimd.dma_start(
                            d_out[:, j0 * 8 : j0 * 8 + wd],
                            vf[:, j0 * 8 : j0 * 8 + wd],
                        )

            # software-pipelined emission: A_k | M_{k-1} | E_{k-2} | C_{k-3}
            for k in range(JL + 3):
                if k < JL:
                    stage_A(k)
                if k == JL and t == 1:
                    # prefetch t=2's first W2 batches during drain-out
                    w2_tiles.clear()
                    ensure_w2(0)
                if 1 <= k < JL + 1:
                    stage_M(k - 1)
                if 2 <= k < JL + 2:
                    stage_E(k - 2)
                if 3 <= k < JL + 3:
                    stage_C(k - 3)

            if t == 1:
                v_prev = v_next


# ---------------------------------------------------------------------------
# Host side
# ---------------------------------------------------------------------------

_NC_CACHE = None


def _get_nc():
    global _NC_CACHE
    if _NC_CACHE is None:
        _NC_CACHE = build_nc()
    return _NC_CACHE


def _host_prep(u, W):
    """Build per-core input maps. u: [B,R,I] f32; W: [1,J,R,O,I] f32."""
    bf = ml_dtypes.bfloat16
    u = np.ascontiguousarray(u, dtype=np.float32)
    Wq = np.ascontiguousarray(W.reshape(J, R, O, I), dtype=np.float32)

    up = np.zeros((B, RP, I), np.float32)
    up[:, :R] = u
    u_ib = np.ascontiguousarray(
        up.transpose(0, 2, 1).reshape(128, I * RP).astype(bf)
    )
    uT = np.ascontiguousarray(u.transpose(1, 2, 0))        # [864, 4, 128]
    uT_ri = np.ascontiguousarray(uT.reshape(C32, 128, 128).astype(bf))
    uTp = np.ascontiguousarray(up.transpose(1, 2, 0))      # [896, 4, 128]
    uTr = np.ascontiguousarray(uTp.reshape(C128, 128, I * B).astype(bf))

    in_maps = []
    for k in range(NCORES):
        j0 = k * JL
        Wk = np.zeros((JL, R, O, I), np.float32)
        real = min(JL, max(0, J - j0))
        if real > 0:
            Wk[:real] = Wq[j0 : j0 + real]
        Wkp = np.zeros((JL, RP, O, I), np.float32)
        Wkp[:, :R] = Wk

        wm_ri = np.ascontiguousarray(
            (Wk / float(R)).transpose(1, 3, 0, 2).reshape(C32, 128, JO).astype(bf)
        )
        wr = np.ascontiguousarray(
            Wkp.transpose(1, 3, 0, 2).reshape(C128, 128, I * JL * O).astype(bf)
        )
        w2 = np.ascontiguousarray(
            Wkp.transpose(0, 2, 3, 1).reshape(JL, 8, I * RP).astype(bf)
        )
        in_maps.append(
            {
                "uT_ri": uT_ri,
                "Wm_ri": wm_ri,
                "Wr": wr,
                "W2": w2,
                "u_ib": u_ib,
                "uTr": uTr,
            }
        )
    return in_maps


def run_cores(u, W, trace=False):
    nc = _get_nc()
    in_maps = _host_prep(u, W)
    res = run_bass_kernel_spmd(
        nc, in_maps, core_ids=list(range(NCORES)), trace=trace
    )
    return res


def kernel(u, W):
    res = run_cores(u, W, trace=False)
    parts = []
    for k in range(NCORES):
        vk = res.results[k]["v_out"]          # [128, JL*O] (b, (j,o))
        parts.append(vk.reshape(B, JL, O))
    full = np.concatenate(parts, axis=1)[:, :J, :]
    return np.ascontiguousarray(full.astype(np.float32))


# revision 4
# speedup vs baseline: 2.6032x; 1.1128x over previous
"""DigitCaps dynamic-routing kernel for 8x TRN2 NeuronCores (Bass/Tile).

Same math as v1, restructured for pipeline overlap:
  - a-phase T1 PSUM drained in 1024-col chunks, rotated across ACT/DVE/Pool
  - i-major T1 layout; i-sum (and +a1 on t=2) via gpsimd accumulate-DMAs
  - exp per-j in b-partition layout on ACT
  - e transposed to r-layout via SP-dispatched DMA transpose
  - cur muls on DVE only; s-phase matmuls in baseline [8,128] form
  - Z via PE ones-matmul on e_all; squash as v1

Sharding: J (166 -> 168 padded) split across 8 cores, 21 capsules each.
"""

import sys

sys.path.insert(0, "/opt/trn_rl_repo")

import numpy as np
import ml_dtypes
from contextlib import ExitStack

import concourse.bacc as bacc
import concourse.bass as bass
import concourse.tile as tile
from concourse import mybir
from concourse.masks import make_identity
from concourse.bass_utils import run_bass_kernel_spmd

F32 = mybir.dt.float32
BF16 = mybir.dt.bfloat16
AX = mybir.AxisListType
ACT_F = mybir.ActivationFunctionType
ALU = mybir.AluOpType

B, J, R, O, I = 128, 166, 864, 8, 4
NCORES = 8
JL = 21          # capsules per core (166 padded to 168)
RP = 896         # R padded to 7*128
C32 = 27         # (r32, i4)=128 contraction chunks over real R=864
C128 = 7         # r chunks of 128 over RP
JO = JL * O      # 168
JB = JL * B      # 2688
NZB = 6          # Z col-blocks of 448 (6*448 = 2688)
ZW = 448
IR = I * RP      # 3584, i-major (i, r) free layout

# engine-assignment knobs (tuned against TimelineSim)
DRAIN_DVE = 0       # j % DRAIN_DVE == DRAIN_DVE-1 -> DVE-fused drain, else ACT
ISUM_POOL = 0       # j % ISUM_POOL == 0 -> Pool accum-DMA i-sum, else DVE adds
CUR_POOL = 0        # j % CUR_POOL == CUR_POOL-1 -> Pool cur muls (0 = never)
YMUL_POOL = 5       # j % YMUL_POOL == YMUL_POOL-1 -> Pool y-mul (0 = never)


def build_nc(repeat=1):
    nc = bacc.Bacc(
        "TRN2", target_bir_lowering=False, debug=False, enable_asserts=False
    )

    d_uT_ri = nc.dram_tensor("uT_ri", [C32, 128, 128], BF16, kind="ExternalInput")
    d_Wm_ri = nc.dram_tensor("Wm_ri", [C32, 128, JO], BF16, kind="ExternalInput")
    d_Wr = nc.dram_tensor("Wr", [C128, 128, I * JL * O], BF16, kind="ExternalInput")
    # W2 in (o, i, r) per j: [JL, 8, IR] with (i major, r minor)
    d_W2 = nc.dram_tensor("W2", [JL, 8, IR], BF16, kind="ExternalInput")
    # u in i-major b-layout: [b, (i, r)]
    d_u_ib = nc.dram_tensor("u_ib", [128, IR], BF16, kind="ExternalInput")
    d_uTr = nc.dram_tensor("uTr", [C128, 128, I * B], BF16, kind="ExternalInput")
    d_out = nc.dram_tensor("v_out", [128, JL * O], F32, kind="ExternalOutput")

    with tile.TileContext(nc) as tc:
        for _ in range(repeat):
            _body(tc, d_uT_ri, d_Wm_ri, d_Wr, d_W2, d_u_ib, d_uTr, d_out)
    nc.compile()
    return nc


def _body(tc, d_uT_ri, d_Wm_ri, d_Wr, d_W2, d_u_ib, d_uTr, d_out):
    nc = tc.nc
    es = ExitStack()
    const = es.enter_context(tc.tile_pool(name="const", bufs=1))
    misc = es.enter_context(tc.tile_pool(name="misc", bufs=1))
    vpool = es.enter_context(tc.tile_pool(name="vpool", bufs=2))
    w2s = es.enter_context(tc.tile_pool(name="w2s", bufs=2))
    ypool = es.enter_context(tc.tile_pool(name="ypool", bufs=3))
    ycpool = es.enter_context(tc.tile_pool(name="ycpool", bufs=6))
    apool = es.enter_context(tc.tile_pool(name="apool", bufs=6))
    epool = es.enter_context(tc.tile_pool(name="epool", bufs=6))
    curp = es.enter_context(tc.tile_pool(name="curp", bufs=16))
    dramp = es.enter_context(tc.tile_pool(name="dramp", bufs=1, space="DRAM"))
    psA = es.enter_context(tc.tile_pool(name="psA", bufs=3, space="PSUM"))
    psB = es.enter_context(tc.tile_pool(name="psB", bufs=2, space="PSUM"))

    with es:
        # ---------------- constants / persistent loads ----------------
        identity = const.tile([128, 128], F32)
        make_identity(nc, identity[:])
        ones_r = const.tile([128, 1], BF16)
        nc.vector.memset(ones_r[:], 1.0)
        ones_r96 = const.tile([128, 1], BF16)
        nc.vector.memset(ones_r96[:], 0.0)
        nc.vector.memset(ones_r96[:96, :], 1.0)
        ones_o = const.tile([8, 1], BF16)
        nc.vector.memset(ones_o[:], 1.0)
        ones18 = const.tile([1, 8], BF16)
        nc.vector.memset(ones18[:], 1.0)

        u_ib = const.tile([128, IR], BF16)
        nc.gpsimd.dma_start(u_ib[:], d_u_ib[:])
        uTr_all = const.tile([128, C128 * I * B], BF16)
        nc.gpsimd.dma_start(
            uTr_all[:].rearrange("p (c n) -> p c n", c=C128),
            d_uTr[:].transpose([1, 0, 2]),
        )
        uTr = [uTr_all[:, c * I * B : (c + 1) * I * B] for c in range(C128)]
        wr_all = const.tile([128, C128 * I * JL * O], BF16)
        nc.gpsimd.dma_start(
            wr_all[:].rearrange("p (c n) -> p c n", c=C128),
            d_Wr[:].transpose([1, 0, 2]),
        )
        wr = [wr_all[:, c * I * JL * O : (c + 1) * I * JL * O] for c in range(C128)]

        g_dram = dramp.tile([1, JB], F32)
        a1_dram = dramp.tile([JL, 128, RP], BF16)

        # ---------------- s0: full (r,i) contraction, b-partition out -------
        _W2_PREFETCH = True
        with tc.tile_pool(name="s0p", bufs=2) as s0p:
            s0ps = psB.tile([128, 512], F32, tag="psB", name="s0ps")
            GB = 9
            for g in range(C32 // GB):
                a = s0p.tile([128, GB * 128], BF16, tag="ut", name=f"ut{g}")
                nc.sync.dma_start(
                    a[:].rearrange("p (c n) -> p c n", c=GB),
                    d_uT_ri[g * GB : (g + 1) * GB].transpose([1, 0, 2]),
                )
                b = s0p.tile([128, GB * JO], BF16, tag="wm", name=f"wm{g}")
                nc.scalar.dma_start(
                    b[:].rearrange("p (c n) -> p c n", c=GB),
                    d_Wm_ri[g * GB : (g + 1) * GB].transpose([1, 0, 2]),
                )
                for cc in range(GB):
                    c = g * GB + cc
                    nc.tensor.matmul(
                        s0ps[:, :JO],
                        lhsT=a[:, cc * 128 : (cc + 1) * 128],
                        rhs=b[:, cc * JO : (cc + 1) * JO],
                        start=(c == 0),
                        stop=(c == C32 - 1),
                    )
            s0_sb = s0p.tile([128, JO], F32)
            nc.scalar.copy(s0_sb[:], s0ps[:, :JO])

            # squash in b-layout (tiny)
            sqb = s0p.tile([128, JO], F32)
            nc.vector.tensor_mul(sqb[:], s0_sb[:], s0_sb[:])
            n2b = s0p.tile([128, JL], F32)
            nc.vector.reduce_sum(
                n2b[:], sqb[:].rearrange("p (j o) -> p j o", j=JL), axis=AX.X
            )
            rtb = s0p.tile([128, JL], F32)
            nc.scalar.sqrt(rtb[:], n2b[:])
            nc.vector.tensor_scalar_add(n2b[:], n2b[:], 1.0)
            nc.vector.reciprocal(n2b[:], n2b[:])
            nc.vector.tensor_mul(rtb[:], rtb[:], n2b[:])  # f = sqrt(n2)/(1+n2)
            v0b = s0p.tile([128, JO], F32)
            nc.vector.tensor_mul(
                v0b[:].rearrange("p (j o) -> p j o", j=JL),
                s0_sb[:].rearrange("p (j o) -> p j o", j=JL),
                rtb[:].unsqueeze(2).broadcast_to([128, JL, O]),
            )
            # transpose v0 to o-partition layout -> per-group v tiles (bf16)
            v_prev = []
            jj = 0
            while jj < JL:
                take = min(4, JL - jj)
                tps = psB.tile([128, 512], F32, tag="psB", name="tps")
                for q in range(take):
                    nc.tensor.transpose(
                        tps[0:8, q * 128 : (q + 1) * 128],
                        v0b[:].rearrange("p (j o) -> p j o", j=JL)[:, jj + q, :],
                        identity[:],
                    )
                vg = vpool.tile([8, 512], BF16, tag=f"vg{jj // 4}", name="vg")
                nc.scalar.copy(vg[:, : take * 128], tps[0:8, : take * 128])
                v_prev.append(vg)
                jj += take

        # ---------------- routing iterations ----------------
        w2_tiles = {}

        def ensure_w2(j):
            b = j // 2
            if b in w2_tiles:
                return w2_tiles[b]
            nb = min(2, JL - j)
            w2 = w2s.tile([8, 2 * IR], BF16, tag="w2", name="w2")
            nc.sync.dma_start(
                w2[:].rearrange("p (j n) -> p j n", j=2)[:, :nb],
                d_W2[j : j + nb].transpose([1, 0, 2]),
            )
            w2_tiles[b] = w2
            return w2

        for t in (1, 2):
            Zt = misc.tile([128, JL], F32, tag="Zt", name=f"Zt_{t}")
            if t == 1:
                v_next = []
                vfin_t = None
            else:
                vfin_t = misc.tile([128, JO], F32, tag="vfin", name="v_fin")
            sps = None
            yc_t = {}
            acc_t = {}
            eT_t = {}
            zsg_t = {}

            def stage_A(j):
                """W2 stream, T1 matmuls, ACT chunk drains."""
                w2 = ensure_w2(j - (j % 2))
                if j % 2 == 0:
                    w2_tiles.pop((j - 2) // 2, None)
                    ensure_w2(j + 2) if j + 2 < JL else None
                w2v = w2[:].rearrange("p (j n) -> p j n", j=2)[:, j % 2]
                lhs = v_prev[j // 4][:, (j % 4) * 128 : (j % 4 + 1) * 128]
                yc = ycpool.tile([128, IR], BF16, tag="yc", name="yc")
                yc_t[j] = yc
                off = 0
                for w in (1024, 1024, 1024, 512):
                    t1 = psA.tile([128, 1024], F32, tag="t1", name="t1")
                    for h in range(0, w, 512):
                        nc.tensor.matmul(
                            t1[:, h : h + 512],
                            lhsT=lhs,
                            rhs=w2v[:, off + h : off + h + 512],
                            start=True,
                            stop=True,
                        )
                    nc.scalar.copy(yc[:, off : off + w], t1[:, :w])
                    off += w

            def stage_M(j):
                """y = T1*u, i-sum adds, a1 store/load."""
                yc = yc_t[j]
                ym_eng = (
                    nc.gpsimd
                    if (YMUL_POOL and j % YMUL_POOL == YMUL_POOL - 1)
                    else nc.vector
                )
                ym_eng.tensor_mul(yc[:], yc[:], u_ib[:])
                a_acc = apool.tile([128, RP], BF16, tag="acc", name="acc")
                acc_t[j] = a_acc
                nc.vector.tensor_add(
                    yc[:, : 2 * RP], yc[:, : 2 * RP], yc[:, 2 * RP :]
                )
                if t == 1:
                    nc.vector.tensor_add(a_acc[:], yc[:, :RP], yc[:, RP : 2 * RP])
                    nc.gpsimd.dma_start(a1_dram[j], a_acc[:])
                else:
                    nc.vector.tensor_add(
                        yc[:, :RP], yc[:, :RP], yc[:, RP : 2 * RP]
                    )
                    a1l = apool.tile([128, RP], BF16, tag="a1l", name="a1l")
                    nc.gpsimd.dma_start(a1l[:], a1_dram[j])
                    nc.vector.tensor_add(a_acc[:], yc[:, :RP], a1l[:])

            def stage_E(j):
                """exp, transpose to r-layout, Z matmuls, rZ."""
                a_acc = acc_t.pop(j)
                e_b = apool.tile([128, RP], BF16, tag="eb", name="e_b")
                # Z accumulates sum_r e (incl. 32 pad cols of exp(0)=1,
                # corrected at the group tail)
                nc.scalar.activation(
                    e_b[:], a_acc[:], ACT_F.Exp, accum_out=Zt[:, j : j + 1]
                )
                e_T = epool.tile([128, C128 * 128], BF16, tag="eT", name="e_T")
                eT_t[j] = e_T
                nc.sync.dma_start_transpose(
                    e_T[:].rearrange("p (c b) -> p c b", c=C128), e_b[:]
                )

            def stage_C(j):
                """cur muls, s-phase matmuls, group tail squash."""
                nonlocal sps
                e_T = eT_t.pop(j)
                if j % 4 == 0:
                    sps = psB.tile([128, 512], F32, tag="psB", name="sps")
                q = j % 4
                cur_eng = (
                    nc.gpsimd
                    if (CUR_POOL and j % CUR_POOL == CUR_POOL - 1)
                    else nc.vector
                )
                curs = []
                for c in range(C128):
                    cur = curp.tile([128, I * B], BF16, tag="cur", name="cur")
                    cur_eng.tensor_mul(
                        cur[:].rearrange("p (i b) -> p i b", i=I),
                        e_T[:, c * 128 : (c + 1) * 128]
                        .unsqueeze(1)
                        .broadcast_to([128, I, B]),
                        uTr[c][:].rearrange("p (i b) -> p i b", i=I),
                    )
                    curs.append(cur)
                for c in range(C128):
                    for i in range(I):
                        nc.tensor.matmul(
                            sps[:, q * 8 : (q + 1) * 8],
                            lhsT=curs[c][:].rearrange("p (i b) -> p i b", i=I)[:, i, :],
                            rhs=wr[c][:, (i * JL + j) * 8 : (i * JL + j) * 8 + 8],
                            start=(c == 0 and i == 0),
                            stop=(c == C128 - 1 and i == I - 1),
                        )
                if q == 3 or j == JL - 1:
                    take = q + 1
                    j0 = j - q
                    wd = take * 8
                    # s_b [128, take*8] from psum; squash in b-layout
                    sb = misc.tile([128, JO], BF16, tag="sb", name=f"sb_{t}")
                    sbs = sb[:, j0 * 8 : j0 * 8 + wd]
                    nc.scalar.copy(sbs, sps[:, :wd])
                    sqb = apool.tile([128, 32], BF16, tag="sqb", name="sqb")
                    nc.scalar.square(sqb[:, :wd], sps[:, :wd])
                    m2b = apool.tile([128, 4], F32, tag="m2b", name="m2b")
                    nc.vector.reduce_sum(
                        m2b[:, :take],
                        sqb[:, :wd].rearrange("p (j o) -> p j o", o=O),
                        axis=AX.X,
                    )
                    gb = apool.tile([128, 4], F32, tag="gb", name="gb")
                    tb = apool.tile([128, 4], F32, tag="tb", name="tb")
                    zc = apool.tile([128, 4], F32, tag="zc", name="zc")
                    # rZ = 1/(Z - 32) ; pad cols contribute exp(0)=1 each
                    nc.vector.tensor_scalar_add(
                        zc[:, :take], Zt[:, j0 : j0 + take], -32.0
                    )
                    nc.vector.reciprocal(zc[:, :take], zc[:, :take])
                    nc.vector.tensor_mul(tb[:, :take], zc[:, :take], zc[:, :take])
                    nc.vector.tensor_mul(
                        m2b[:, :take], m2b[:, :take], tb[:, :take]
                    )  # q
                    nc.scalar.sqrt(tb[:, :take], m2b[:, :take])  # sqrt(q)
                    nc.vector.tensor_scalar_add(m2b[:, :take], m2b[:, :take], 1.0)
                    nc.vector.reciprocal(m2b[:, :take], m2b[:, :take])  # 1/(1+q)
                    nc.vector.tensor_mul(tb[:, :take], tb[:, :take], m2b[:, :take])
                    nc.vector.tensor_mul(
                        gb[:, :take], tb[:, :take], zc[:, :take]
                    )  # g
                    if t == 1:
                        # v_b = s_b * g  then transpose to o-layout group tile
                        vb = apool.tile([128, 32], F32, tag="vb", name="vb")
                        nc.vector.tensor_mul(
                            vb[:, :wd].rearrange("p (j o) -> p j o", o=O),
                            sbs.rearrange("p (j o) -> p j o", o=O),
                            gb[:, :take].unsqueeze(2).broadcast_to(
                                [128, take, O]
                            ),
                        )
                        tps = psB.tile([128, 512], F32, tag="psB", name="tpv")
                        for qq in range(take):
                            nc.tensor.transpose(
                                tps[0:8, qq * 128 : (qq + 1) * 128],
                                vb[:, qq * 8 : (qq + 1) * 8],
                                identity[:],
                            )
                        vg = vpool.tile([8, 512], BF16, tag=f"vg{j0 // 4}",
                                        name="vgn")
                        nc.scalar.copy(vg[:, : take * 128], tps[0:8, : take * 128])
                        v_next.append(vg)
                    else:
                        vf = vfin_t
                        nc.vector.tensor_mul(
                            vf[:, j0 * 8 : j0 * 8 + wd].rearrange(
                                "p (j o) -> p j o", o=O
                            ),
                            sbs.rearrange("p (j o) -> p j o", o=O),
                            gb[:, :take].unsqueeze(2).broadcast_to(
                                [128, take, O]
                            ),
                        )
                        nc.gpsimd.dma_start(
                            d_out[:, j0 * 8 : j0 * 8 + wd],
                            vf[:, j0 * 8 : j0 * 8 + wd],
                        )

            # software-pipelined emission: A_k | M_{k-1} | E_{k-2} | C_{k-3}
            for k in range(JL + 3):
                if k < JL:
                    stage_A(k)
                if k == JL and t == 1:
                    # prefetch t=2's first W2 batches during drain-out
                    w2_tiles.clear()
                    ensure_w2(0)
                if 1 <= k < JL + 1:
                    stage_M(k - 1)
                if 2 <= k < JL + 2:
                    stage_E(k - 2)
                if 3 <= k < JL + 3:
                    stage_C(k - 3)

            if t == 1:
                v_prev = v_next


# ---------------------------------------------------------------------------
# Host side
# ---------------------------------------------------------------------------

_NC_CACHE = None


def _get_nc():
    global _NC_CACHE
    if _NC_CACHE is None:
        _NC_CACHE = build_nc()
    return _NC_CACHE


def _host_prep(u, W):
    """Build per-core input maps. u: [B,R,I] f32; W: [1,J,R,O,I] f32."""
    bf = ml_dtypes.bfloat16
    u = np.ascontiguousarray(u, dtype=np.float32)
    Wq = np.ascontiguousarray(W.reshape(J, R, O, I), dtype=np.float32)

    up = np.zeros((B, RP, I), np.float32)
    up[:, :R] = u
    u_ib = np.ascontiguousarray(
        up.transpose(0, 2, 1).reshape(128, I * RP).astype(bf)
    )
    uT = np.ascontiguousarray(u.transpose(1, 2, 0))        # [864, 4, 128]
    uT_ri = np.ascontiguousarray(uT.reshape(C32, 128, 128).astype(bf))
    uTp = np.ascontiguousarray(up.transpose(1, 2, 0))      # [896, 4, 128]
    uTr = np.ascontiguousarray(uTp.reshape(C128, 128, I * B).astype(bf))

    in_maps = []
    for k in range(NCORES):
        j0 = k * JL
        Wk = np.zeros((JL, R, O, I), np.float32)
        real = min(JL, max(0, J - j0))
        if real > 0:
            Wk[:real] = Wq[j0 : j0 + real]
        Wkp = np.zeros((JL, RP, O, I), np.float32)
        Wkp[:, :R] = Wk

        wm_ri = np.ascontiguousarray(
            (Wk / float(R)).transpose(1, 3, 0, 2).reshape(C32, 128, JO).astype(bf)
        )
        wr = np.ascontiguousarray(
            Wkp.transpose(1, 3, 0, 2).reshape(C128, 128, I * JL * O).astype(bf)
        )
        w2 = np.ascontiguousarray(
            Wkp.transpose(0, 2, 3, 1).reshape(JL, 8, I * RP).astype(bf)
        )
        in_maps.append(
            {
                "uT_ri": uT_ri,
                "Wm_ri": wm_ri,
                "Wr": wr,
                "W2": w2,
                "u_ib": u_ib,
                "uTr": uTr,
            }
        )
    return in_maps


def run_cores(u, W, trace=False):
    nc = _get_nc()
    in_maps = _host_prep(u, W)
    res = run_bass_kernel_spmd(
        nc, in_maps, core_ids=list(range(NCORES)), trace=trace
    )
    return res


def kernel(u, W):
    res = run_cores(u, W, trace=False)
    parts = []
    for k in range(NCORES):
        vk = res.results[k]["v_out"]          # [128, JL*O] (b, (j,o))
        parts.append(vk.reshape(B, JL, O))
    full = np.concatenate(parts, axis=1)[:, :J, :]
    return np.ascontiguousarray(full.astype(np.float32))
